# revision 25
# baseline (speedup 1.0000x reference)
"""Trainium2 Bass kernel for nn_MultiHeadMLPAttentionModel (v2).

Model (per b, per point n): pairwise = [radar_b(4), pt(2)]; radar constant
over n folds into per-(b,head) biases computed on host.

  sh    = relu(Wsc pt + cb)      score-net hidden, 4 heads x 64   [N,256]
  logit = w2 . sh                per head                          [N,4]
  h1    = relu(We pt + cbe)      encoder hidden                    [N,64]
  w     = softmax(logit, n);  ctx = sum_n w * h1  (then enc2+MLP on pooled)

v2 design (calibrated on HW microbenchmarks):
  * inputs + layer-1 weights in fp8e4 with hi/lo weight splitting; host-sim
    rel err 2.8e-3 vs fp32 reference (tolerance 2e-2).
  * score-hidden production: [K=8,M=128,N=512] matmuls row-packed 4x via
    tile_position=(32j,0) -> ~107ns each (4 concurrent PE row-groups).
    Data is DMA-replicated at partition offsets 0/32/64/96 to feed them.
  * logits: n-major drains, lhsT = sh-block [128,128] (FWL weight loads),
    rhs = w2 columns [128,2]; output lands points-major so softmax-exp
    output feeds pooling with no transposes.
  * pooling: lhsT = exp-weights [128pts,4heads], rhs = h1n [128,65];
    4 batch rows col-packed per PSUM bank via tile_position=(0,32r);
    accumulated across all 64 point-blocks in PSUM.
  * relu-copies (the throughput floor) split across DVE and ACT every unit.
  * 2 passes x 8 batch rows to fit PSUM (3 sh + 1 lg + 2 h1 + 2 ctx = 8 banks).

Sharding: pure data parallel over B: 8 cores x 16 rows.
"""

import numpy as np

import concourse.bass as bass
import concourse.tile as tile
from concourse import bacc, mybir

B, N, HID, HEADS = 128, 8192, 64, 4
NCORES = 8
BPC = B // NCORES      # 16 batch rows per core
CHUNK = 512
NCH = N // CHUNK       # 16
PASSES = 4
BPP = BPC // PASSES    # batch rows per pass
import os
ROWPACK = os.environ.get("KV2_ROWPACK", "1") == "1"
ROWPACK_H1 = os.environ.get("KV2_ROWPACK_H1", "0") == "1"
COLPACK = os.environ.get("KV2_COLPACK", "1") == "1"

F32 = mybir.dt.float32
BF16 = mybir.dt.bfloat16
FP8 = mybir.dt.float8e4
AF = mybir.ActivationFunctionType
ALU = mybir.AluOpType


def build_nc():
    from contextlib import ExitStack

    nc = bacc.Bacc()
    f32 = F32

    # point data, n-major, 4 partition-group replicas of 8 rows
    xpn_d = nc.dram_tensor("xpn", [PASSES, 4, 8, NCH * BPP * CHUNK], FP8,
                           kind="ExternalInput")
    wsc_d = nc.dram_tensor("wsc", [4, 8, BPC * 2 * 128], FP8, kind="ExternalInput")
    wenc_d = nc.dram_tensor("wenc", [4, 8, BPC * 65], FP8, kind="ExternalInput")
    w2n_d = nc.dram_tensor("w2n", [128, 4], BF16, kind="ExternalInput")
    ew2b_d = nc.dram_tensor("ew2b", [65, 64], f32, kind="ExternalInput")
    ow1_d = nc.dram_tensor("ow1", [64, 256], f32, kind="ExternalInput")
    ob1_d = nc.dram_tensor("ob1", [1, 64], f32, kind="ExternalInput")
    w2o_d = nc.dram_tensor("w2o", [65, 1], f32, kind="ExternalInput")
    id64f_d = nc.dram_tensor("id64f", [64, 64], f32, kind="ExternalInput")
    on16_d = nc.dram_tensor("on16", [1, BPC], f32, kind="ExternalInput")
    out_d = nc.dram_tensor("out", [BPC], f32, kind="ExternalOutput")

    with tile.TileContext(nc) as tc, ExitStack() as ctx:
        consts = ctx.enter_context(tc.tile_pool(name="consts", bufs=1))

        wsc_s = consts.tile([128, BPC * 2 * 128], FP8, name="wsc_s", tag="wsc_s")
        wenc_s = consts.tile([128, BPC * 65], FP8, name="wenc_s", tag="wenc_s")
        for j in range(4):
            nc.sync.dma_start(wsc_s[32 * j : 32 * j + 8, :], wsc_d[j])
            nc.sync.dma_start(wenc_s[32 * j : 32 * j + 8, :], wenc_d[j])



        def cload(dram, shape, nm, dt=f32):
            t = consts.tile(shape, dt, name=nm, tag=nm)
            nc.sync.dma_start(t[:], dram[:])
            return t

        w2n_s = cload(w2n_d, [128, 4], "w2n_s", BF16)
        ew2b_s = cload(ew2b_d, [65, 64], "ew2b_s")
        ow1_s = cload(ow1_d, [64, 256], "ow1_s")
        ob1_s = cload(ob1_d, [1, 64], "ob1_s")
        w2o_s = cload(w2o_d, [65, 1], "w2o_s")
        id64f_s = cload(id64f_d, [64, 64], "id64f_s")
        on16_s = cload(on16_d, [1, BPC], "on16_s")

        ctxnT = consts.tile([65, 64], f32, name="ctxnT", tag="ctxnT")
        obuf = consts.tile([65, BPC], f32, name="obuf", tag="obuf")
        fct = consts.tile([64, 64], f32, name="fct", tag="fct")
        res = consts.tile([1, BPC], f32, name="res", tag="res")
        nc.vector.memset(ctxnT[64:65, :], 1.0)
        nc.vector.memset(obuf[64:65, :], 1.0)

        for p in range(PASSES):
            with ExitStack() as passctx:
                ctxps = passctx.enter_context(
                    tc.tile_pool(name=f"ctxps{p}", bufs=1, space="PSUM"))
                ctx_t = [ctxps.tile([128, 65], f32, name="ctx0", tag="ctx0")]

                with ExitStack() as cctx:
                    xpool = cctx.enter_context(tc.tile_pool(name="xp", bufs=4))
                    shsb = cctx.enter_context(tc.tile_pool(name="shsb", bufs=5))
                    h1sb = cctx.enter_context(tc.tile_pool(name="h1sb", bufs=2))
                    esb = cctx.enter_context(tc.tile_pool(name="esb", bufs=2))
                    shps = cctx.enter_context(
                        tc.tile_pool(name="shps", bufs=2, space="PSUM"))
                    lgps = cctx.enter_context(
                        tc.tile_pool(name="lgps", bufs=1, space="PSUM"))
                    h1ps = cctx.enter_context(
                        tc.tile_pool(name="h1ps", bufs=2, space="PSUM"))

                    xqs = {}

                    def load_x(c, p=p):
                        t = xpool.tile([128, BPP * CHUNK], FP8, name="xq",
                                       tag="xq")
                        for j in range(4):
                            nc.gpsimd.dma_start(
                                t[32 * j : 32 * j + 8, :],
                                xpn_d[p, j, :, c * BPP * CHUNK
                                      : (c + 1) * BPP * CHUNK])
                        xqs[c] = t

                    for _pf in range(min(3, NCH)):
                        load_x(_pf)

                    # HAM warmup: ~4.5us of dense back-to-back matmuls flips the
                    # PE clock-gate to K=8/8 (2.4 GHz); it only re-throttles after
                    # a fully-idle 3.4us window, which never occurs mid-pass.
                    warm = shps.tile([128, 1024], f32, name="warm", tag="sh")
                    for _w in range(2):
                        nc.tensor.matmul(
                            warm[:, 0:512], wsc_s[0:8, 0:128], wsc_s[0:8, 0:512],
                            start=True, stop=True, skip_group_check=True)

                    e_prev = None
                    h1_prev = None
                    VS = BPP      # full units of 512 points
                    D = 2         # deferred-slot pipeline depth

                    for c in range(NCH + 1):
                        last = c == NCH
                        if not last:
                            if c + 3 < NCH:
                                load_x(c + 3)
                            xq = xqs.pop(c)
                            co = 0
                            lg = lgps.tile([128, BPP * 16], f32, name="lg",
                                           tag="lg")
                            h1_cur = [None] * BPP
                            h1p_cur = [None] * BPP
                            shs_q = {}

                        def pools_prev(v, cp=c - 1):
                            # 4 pool matmuls for unit v of the previous chunk
                            u = v
                            r = u % 4
                            for t in range(4):
                                ec = u * 16 + t * 4
                                nc.tensor.matmul(
                                    ctx_t[0][32 * r : 32 * r + 4, :],
                                    e_prev[:, ec : ec + 4],
                                    h1_prev[u][:, 65 * t : 65 * t + 65],
                                    start=(cp == 0 and t == 0),
                                    stop=(cp == NCH - 1 and t == 3),
                                    tile_position=(0, 32 * r) if COLPACK else None,
                                    skip_group_check=True,
                                )

                        def deferred(v):
                            u = v
                            bb = p * BPP + u
                            shs = shs_q.pop(v)
                            for t in range(4):
                                for hp in range(2):
                                    lc = u * 16 + t * 4 + hp * 2
                                    nc.tensor.matmul(
                                        lg[:, lc : lc + 2],
                                        shs[:, hp * 512 + t * 128
                                            : hp * 512 + t * 128 + 128],
                                        w2n_s[:, hp * 2 : hp * 2 + 2],
                                        start=True, stop=True,
                                        skip_group_check=True,
                                    )
                            h1p_cur[u] = h1ps.tile([128, 260], f32, name="h1p",
                                                   tag="h1p")
                            gj = u % 4 if ROWPACK else 0
                            rj = 32 * gj
                            for t in range(4):
                                nc.tensor.matmul(
                                    h1p_cur[u][:, 65 * t : 65 * t + 65],
                                    xq[rj : rj + 8,
                                       co + u * CHUNK + t * 128
                                       : co + u * CHUNK + t * 128 + 128],
                                    wenc_s[rj : rj + 8,
                                           bb * 65 : bb * 65 + 65],
                                    start=True, stop=True,
                                    tile_position=(32 * gj, 0) if ROWPACK
                                    else None,
                                    skip_group_check=True,
                                )
                            h1s = h1sb.tile([128, 260], BF16, name="h1s",
                                            tag=f"h1_{u}")
                            if u in (0, BPP - 1):
                                nc.vector.tensor_scalar(
                                    h1s[:], h1p_cur[u][:], 0.0, None, ALU.max)
                            else:
                                nc.scalar.activation(h1s[:], h1p_cur[u][:],
                                                     AF.Relu)
                            h1_cur[u] = h1s
                            if e_prev is not None:
                                pools_prev(v)

                        if last:
                            if e_prev is not None:
                                for v in range(VS):
                                    pools_prev(v)
                            break

                        for v in range(VS + D):
                            if v < VS:
                                u = v
                                bb = p * BPP + u
                                sh = shps.tile([128, 1024], f32, name="sh",
                                               tag="sh")
                                for hp in range(2):
                                    g = (2 * u + hp) % 4 if ROWPACK else 0
                                    cw = (bb * 2 + hp) * 128
                                    cs = co + u * CHUNK
                                    ro = 32 * g
                                    nc.tensor.matmul(
                                        sh[:, hp * 512 : (hp + 1) * 512],
                                        wsc_s[ro : ro + 8, cw : cw + 128],
                                        xq[ro : ro + 8, cs : cs + 512],
                                        start=True, stop=True,
                                        tile_position=(32 * g, 0) if ROWPACK
                                        else None,
                                        skip_group_check=True,
                                    )
                                shs = shsb.tile([128, 1024], BF16, name="shs",
                                                tag="shs")
                                nc.vector.tensor_scalar(
                                    shs[:, 0:512], sh[:, 0:512], 0.0, None,
                                    ALU.max)
                                nc.scalar.activation(
                                    shs[:, 512:1024], sh[:, 512:1024], AF.Relu)
                                shs_q[v] = shs
                            if v >= D:
                                deferred(v - D)

                        e = esb.tile([128, BPP * 16], BF16, name="e", tag="e")
                        nc.scalar.activation(e[:], lg[:], AF.Exp)
                        e_prev = e
                        h1_prev = h1_cur

                # --- normalize + transpose ctx for this pass ---
                with ExitStack() as nctx:
                    smpool = nctx.enter_context(tc.tile_pool(name="smp", bufs=2))
                    psU = nctx.enter_context(
                        tc.tile_pool(name="psU", bufs=2, space="PSUM"))
                    for u in range(BPP):
                        b = p * BPP + u
                        r = u % 4
                        rows = slice(32 * r, 32 * r + 4)
                        rz = smpool.tile([4, 1], f32, name="rz", tag="rz")
                        nc.vector.reciprocal(rz[:], ctx_t[0][rows, 64:65])
                        ctxn = smpool.tile([4, 64], f32, name="ctxn", tag="ctxn")
                        nc.vector.tensor_scalar_mul(
                            ctxn[:], ctx_t[0][rows, 0:64], rz[:])
                        tp_ps = psU.tile([64, 4], f32, name="tp_ps", tag="tp2")
                        nc.tensor.transpose(tp_ps[:], ctxn[:], id64f_s[0:4, 0:4])
                        nc.vector.tensor_copy(
                            out=ctxnT[0:64, b * 4 : (b + 1) * 4], in_=tp_ps[:])

        # ---- Phase D: pooled-context encoder layer 2 + output MLP ----
        with ExitStack() as pctx:
            psD = pctx.enter_context(tc.tile_pool(name="psD", bufs=1, space="PSUM"))
            fct_ps = psD.tile([64, 64], f32, name="fct_ps", tag="fctp")
            nc.tensor.matmul(fct_ps[:], ew2b_s[:], ctxnT[:], start=True, stop=True)
            nc.vector.tensor_copy(out=fct[:], in_=fct_ps[:])
            fct_bh = fct.rearrange("d (b h) -> d b h", h=HEADS)
            o1_ps = psD.tile([64, BPC], f32, name="o1_ps", tag="o1p")
            for h in range(HEADS):
                nc.tensor.matmul(
                    o1_ps[:],
                    ow1_s[:, h * 64 : (h + 1) * 64],
                    fct_bh[:, :, h],
                    start=(h == 0),
                    stop=False,
                    skip_group_check=True,
                )
            nc.tensor.matmul(
                o1_ps[:], ob1_s[:], on16_s[:], start=False, stop=True,
                skip_group_check=True,
            )
            nc.scalar.activation(obuf[0:64, :], o1_ps[:], AF.Relu)
            fin_ps = psD.tile([1, BPC], f32, name="fin_ps", tag="finp")
            nc.tensor.matmul(fin_ps[:], w2o_s[:], obuf[:], start=True, stop=True)
            nc.vector.tensor_copy(out=res[:], in_=fin_ps[:])
            nc.sync.dma_start(out_d.rearrange("(a n) -> a n", a=1), res[:])

    if not nc.is_finalized():
        nc.finalize()
    return nc


def make_in_maps(inputs):
    """Host-side marshalling: fp8 data/weights with hi/lo weight rows.

    Row scheme (8 rows, paired data x weights):
      data:    [xh, yh, xl, yl, xh, yh, 1, 1]
      weights: [wxh, wyh, wxh, wyh, wxl, wyl, bh, bl]
    so the product accumulates wxh*xh + wyh*yh + wxh*xl + wyh*yl + wxl*xh
    + wyl*yh + bh + bl  ~= wx*x + wy*y + bias at ~7-bit mantissa."""
    import ml_dtypes

    f8 = ml_dtypes.float8_e4m3
    f = np.float32

    def split8(a):
        hi = a.astype(f8)
        lo = (a - hi.astype(f)).astype(f8)
        return hi.astype(f), lo.astype(f)

    radar = np.concatenate(
        [np.asarray(inputs["radar_xy"], f), np.asarray(inputs["radar_dir"], f)],
        axis=1)
    pts = np.asarray(inputs["pts"], f)
    enc_w1 = np.asarray(inputs["enc_w1"], f)
    enc_b1 = np.asarray(inputs["enc_b1"], f)
    enc_w2 = np.asarray(inputs["enc_w2"], f)
    enc_b2 = np.asarray(inputs["enc_b2"], f)
    sc_w1 = np.asarray(inputs["sc_w1"], f)
    sc_b1 = np.asarray(inputs["sc_b1"], f)
    sc_w2 = np.asarray(inputs["sc_w2"], f)
    out_w1 = np.asarray(inputs["out_w1"], f)
    out_b1 = np.asarray(inputs["out_b1"], f)
    out_w2 = np.asarray(inputs["out_w2"], f)
    out_b2 = np.asarray(inputs["out_b2"], f)

    cb_sc = np.einsum("br,hrd->bhd", radar, sc_w1[:, :4, :]) + sc_b1  # [B,4,64]
    cb_enc = radar @ enc_w1[:4] + enc_b1                              # [B,64]

    # fp8 hi/lo of point coords, data rows [xh, yh, xl, yl, xh, yh, 1, 1]
    xh, xl = split8(pts[:, :, 0])
    yh, yl = split8(pts[:, :, 1])
    xrows = np.stack([xh, yh, xl, yl, xh, yh,
                      np.ones_like(xh), np.ones_like(xh)], axis=1)  # [B,8,N]

    def wrows(wx, wy, bias):
        # -> [8, 64] f32: [wxh, wyh, wxh, wyh, wxl, wyl, bh, bl]
        wxh, wxl = split8(wx)
        wyh, wyl = split8(wy)
        bh, bl = split8(bias)
        return np.stack([wxh, wyh, wxh, wyh, wxl, wyl, bh, bl], axis=0)

    ew2b = np.concatenate([enc_w2, enc_b2[None, :]], axis=0)
    ow1 = np.empty((64, 256), f)
    for h in range(HEADS):
        ow1[:, h * 64 : (h + 1) * 64] = out_w1[h * 64 : (h + 1) * 64, :]
    ob1 = np.ascontiguousarray(out_b1[None, :])
    w2o = np.concatenate([out_w2, out_b2[None, :]], axis=0)
    id64f = np.eye(64, dtype=f)
    on16 = np.ones((1, BPC), f)

    bf = ml_dtypes.bfloat16
    w2n = np.zeros((128, 4), bf)
    w2n[0:64, 0] = sc_w2[0]
    w2n[64:128, 1] = sc_w2[1]
    w2n[0:64, 2] = sc_w2[2]
    w2n[64:128, 3] = sc_w2[3]

    in_maps = []
    for core in range(NCORES):
        sl = slice(core * BPC, (core + 1) * BPC)
        # xpn [PASSES, NCH, 4, 8, BPP*CHUNK]
        xr = xrows[sl]  # [16, 8, N]
        xpn = np.empty((PASSES, 4, 8, NCH * BPP * CHUNK), f8)
        for p in range(PASSES):
            for c in range(NCH):
                seg = xr[p * BPP : (p + 1) * BPP, :,
                         c * CHUNK : (c + 1) * CHUNK]      # [4b, 8rows, 512]
                flat = seg.transpose(1, 0, 2).reshape(8, BPP * CHUNK).astype(f8)
                xpn[p, :, :, c * BPP * CHUNK : (c + 1) * BPP * CHUNK] = flat
        # wsc [4, 8, BPC*2*128]
        wsc = np.zeros((8, BPC * 2 * 128), f)
        wenc = np.zeros((8, BPC * 65), f)
        for bl in range(BPC):
            b = core * BPC + bl
            for hp in range(2):
                w8 = np.concatenate(
                    [wrows(sc_w1[2 * hp + i, 4], sc_w1[2 * hp + i, 5],
                           cb_sc[b, 2 * hp + i]) for i in range(2)],
                    axis=1)  # [8, 128]
                wsc[:, (bl * 2 + hp) * 128 : (bl * 2 + hp + 1) * 128] = w8
            wenc[:, bl * 65 : bl * 65 + 64] = wrows(enc_w1[4], enc_w1[5],
                                                    cb_enc[b])
            wenc[6, bl * 65 + 64] = 1.0  # normalizer column
        wsc8 = np.broadcast_to(wsc.astype(f8), (4, 8, BPC * 2 * 128)).copy()
        wenc8 = np.broadcast_to(wenc.astype(f8), (4, 8, BPC * 65)).copy()
        in_maps.append(
            dict(xpn=xpn, wsc=wsc8, wenc=wenc8, w2n=w2n, ew2b=ew2b, ow1=ow1,
                 ob1=ob1, w2o=w2o, id64f=id64f, on16=on16))
    return in_maps


_CACHE = {}


def _get_runner():
    """Build the Bass program once and a cached jitted PJRT executable over
    the 8 cores (shard_map along axis 0 of every input)."""
    if "runner" in _CACHE:
        return _CACHE["runner"]

    import jax
    from jax.sharding import Mesh, NamedSharding, PartitionSpec

    from concourse.bass2jax import (
        _bass_exec_p,
        install_neuronx_cc_hook,
        partition_id_tensor,
        shard_map,
    )

    nc = build_nc()
    _CACHE["nc"] = nc
    install_neuronx_cc_hook()
    partition_name = nc.partition_id_tensor.name if nc.partition_id_tensor else None
    in_names, out_names, out_avals = [], [], []
    for alloc in nc.m.functions[0].allocations:
        if not isinstance(alloc, mybir.MemoryLocationSet):
            continue
        name = alloc.memorylocations[0].name
        if alloc.kind == "ExternalInput":
            if name != partition_name:
                in_names.append(name)
        elif alloc.kind == "ExternalOutput":
            out_names.append(name)
            out_avals.append(
                jax.core.ShapedArray(tuple(alloc.tensor_shape), mybir.dt.np(alloc.dtype))
            )
    all_in_names = tuple(in_names + out_names)
    if partition_name is not None:
        all_in_names = all_in_names + (partition_name,)

    def _body(*args):
        operands = list(args)
        if partition_name is not None:
            operands.append(partition_id_tensor())
        return tuple(
            _bass_exec_p.bind(
                *operands,
                out_avals=tuple(out_avals),
                in_names=all_in_names,
                out_names=tuple(out_names),
                lowering_input_output_aliases=(),
                sim_require_finite=True,
                sim_require_nnan=True,
                nc=nc,
            )
        )

    devices = jax.devices()[:NCORES]
    mesh = Mesh(np.asarray(devices), ("core",))
    nin = len(in_names) + len(out_names)
    fn = jax.jit(
        shard_map(
            _body,
            mesh=mesh,
            in_specs=(PartitionSpec("core"),) * nin,
            out_specs=(PartitionSpec("core"),) * len(out_names),
            check_rep=False,
        ),
        keep_unused=True,
    )
    sharding = NamedSharding(mesh, PartitionSpec("core"))
    runner = (fn, sharding, in_names, out_avals)
    _CACHE["runner"] = runner
    return runner


def kernel(**inputs):
    import jax

    in_maps = make_in_maps(inputs)
    fn, sharding, in_names, out_avals = _get_runner()
    concat_in = [
        np.concatenate([np.asarray(in_maps[c][name]) for c in range(NCORES)], axis=0)
        for name in in_names
    ]
    concat_zeros = [
        np.zeros((NCORES * a.shape[0], *a.shape[1:]), a.dtype) for a in out_avals
    ]
    args = [jax.device_put(a, sharding) for a in (*concat_in, *concat_zeros)]
    (out,) = fn(*args)
    return np.asarray(out).reshape(B).astype(np.float32)
